# revision 47
# baseline (speedup 1.0000x reference)
"""Bass/Trainium2 kernel for DropConnect (training path, Wstd != 0).

Z[b,o] = sum_i X[b,i] * W[i,o] * Werr[loc_id[b],i,o] + bias[o] * Berr[loc_id[b],o]

Strategy (8 NeuronCores, data-parallel over batch):
  - each core handles 16 samples; loc_id is known on the host at launch, so
    the per-sample Werr/Berr rows are gathered host-side while sharding and
    shipped per-core as plain contiguous inputs (the "all-gather of the
    needed rows" sharding choice) -- no on-device indirect DMA at all
  - slabs are shipped in bf16 (tolerance is 2e-2; measured end-to-end
    rel err ~3e-3), halving HBM traffic to ~8.4 MB/core
  - W and the whole slab stream ride ONE HWDGE ring (SP) in consumption
    order: per-ring FIFO guarantees arrival order with no packet-size
    competition (the DMA sustains ~425 GB/s after a ~5us ramp); the tiny
    xt/bias/berr inputs drain on the ACT ring
  - VectorE multiplies slab pairs with W in fused [128,4096] ops (2x_1P
    bf16 mode) -- the steady-state pacer at ~1.14us/sample; head/tail
    slabs are halved so compute starts/ends tighter against the stream
  - TensorE contracts with X: per sample, 4 accumulating [128,1]x[128,512]
    matmuls into a [1,512] PSUM tile (215ns issue cadence warm)
  - bias*Berr is computed on the otherwise-idle GpSimd and pre-stored to
    the output; ScalarE copies each PSUM row into a flat staging tile and
    the group stores ACCUMULATE onto the pre-store via the SDMA CCE adder
    (single SWDGE queue FIFO guarantees ordering)
"""

import sys

sys.path.insert(0, "/opt/trn_rl_repo")

import ml_dtypes
import numpy as np

B, IN, OUT, POOL, NCORES = 128, 512, 512, 1000, 8
BL = B // NCORES  # samples per core
WT_COLS = 4 * OUT  # 2048: one macro-row = 4 input rows of W/Werr

BF16 = ml_dtypes.bfloat16

# slab -> chunk grouping: two 1-slab chunks first (fast first compute),
# 1MB pair chunks in the middle, single-slab chunks last (short drain).
CHUNK_SLABS = [[0], [1], [2, 3], [4, 5], [6, 7], [8, 9], [10, 11], [12, 13], [14], [15]]

# sample index -> (first sample, count) for the accumulating output stores;
# the final two stores cover 2 samples each -- more stores than this loses:
# each SWDGE store costs ~0.6-0.9us of serialized Q7 issue time
STORE_AT = {3: (0, 4), 7: (4, 4), 11: (8, 4), 13: (12, 2), 15: (14, 2)}

_CACHE = {}


def _build():
    import concourse.mybir as mybir
    import concourse.tile as tile
    from concourse import bacc

    f32, bf16 = mybir.dt.float32, mybir.dt.bfloat16

    nc = bacc.Bacc("TRN2", debug=False)
    wd = nc.dram_tensor("WD", [128, BL * WT_COLS], bf16, kind="ExternalInput")
    wr = nc.dram_tensor("Wr", [128, WT_COLS], bf16, kind="ExternalInput")
    xt = nc.dram_tensor("Xt", [128, BL * 4], bf16, kind="ExternalInput")
    bias16 = nc.dram_tensor("bias16", [BL, OUT], f32, kind="ExternalInput")
    berr16 = nc.dram_tensor("berr16", [BL, OUT], f32, kind="ExternalInput")
    z = nc.dram_tensor("Z", [1, BL * OUT], f32, kind="ExternalOutput")

    with tile.TileContext(nc) as tc:
        with (
            tc.tile_pool(name="const", bufs=1) as cpool,
            tc.tile_pool(name="wts", bufs=8) as wpool,
            tc.tile_pool(name="prod", bufs=6) as ptpool,
            tc.tile_pool(name="ps", bufs=8, space="PSUM") as ppool,
        ):
            wt_tiles = {}

            def chunk_dma(ci, ring):
                slabs = CHUNK_SLABS[ci]
                w = len(slabs) * WT_COLS
                t = wpool.tile([128, 2 * WT_COLS], bf16, tag="wt")
                base = slabs[0] * WT_COLS
                if ci >= len(CHUNK_SLABS) - 2:
                    # tail slabs arrive as two half-DMAs so their multiplies
                    # and matmuls start before the full slab lands
                    h = w // 2
                    ring.dma_start(t[:, :h], wd.ap()[:, base : base + h])
                    ring.dma_start(t[:, h:w], wd.ap()[:, base + h : base + w])
                else:
                    ring.dma_start(t[:, :w], wd.ap()[:, base : base + w])
                wt_tiles[ci] = t

            # W and the ENTIRE slab stream ride the SP ring: FIFO order per
            # ring means W beats the flood and slabs arrive exactly in
            # consumption order with no packet competition. The tiny
            # xt/bias/berr inputs drain on the ACT ring; only the (idle)
            # GpSimd memb product and the first matmul's stationary read
            # depend on them.
            wr_sb = cpool.tile([128, WT_COLS], bf16)
            nc.sync.dma_start(wr_sb[:], wr.ap())
            xt_sb = cpool.tile([128, BL * 4], bf16)
            nc.scalar.dma_start(xt_sb[:], xt.ap())
            bias_sb = cpool.tile([BL, OUT], f32)
            nc.scalar.dma_start(bias_sb[:], bias16.ap())
            berr_sb = cpool.tile([BL, OUT], f32)
            nc.scalar.dma_start(berr_sb[:], berr16.ap())
            for ci in range(len(CHUNK_SLABS)):
                chunk_dma(ci, nc.sync)

            memb_sb = cpool.tile([BL, OUT], bf16)
            zstage = cpool.tile([1, BL * OUT], f32)

            for ci, slabs in enumerate(CHUNK_SLABS):
                wt = wt_tiles[ci]
                w = len(slabs) * WT_COLS
                pt = ptpool.tile([128, 2 * WT_COLS], bf16, tag="pt")
                if ci >= len(CHUNK_SLABS) - 2:
                    h = w // 2
                    nc.vector.tensor_mul(pt[:, :h], wt[:, :h], wr_sb[:, :h])
                    nc.vector.tensor_mul(pt[:, h:w], wt[:, h:w], wr_sb[:, h:w])
                elif len(slabs) == 1:
                    nc.vector.tensor_mul(pt[:, :w], wt[:, :w], wr_sb[:])
                else:
                    nc.vector.tensor_mul(pt[:, :w], wt[:, :w], wr2_sb[:])
                for si, b in enumerate(slabs):
                    ps = ppool.tile([1, OUT], f32, tag="ps")
                    for j in range(4):
                        nc.tensor.matmul(
                            out=ps[:],
                            lhsT=xt_sb[:, 4 * b + j : 4 * b + j + 1],
                            rhs=pt[:, si * WT_COLS + j * OUT : si * WT_COLS + (j + 1) * OUT],
                            start=(j == 0),
                            stop=(j == 3),
                        )
                    nc.scalar.copy(out=zstage[0:1, b * OUT : (b + 1) * OUT], in_=ps[:])
                    # accumulate finished rows onto the memb pre-store; the
                    # final two stores cover 2 samples each so the last store
                    # is small and issues as early as possible
                    if b in STORE_AT:
                        s0, n = STORE_AT[b]
                        nc.gpsimd.dma_start(
                            z.ap()[:, s0 * OUT : (s0 + n) * OUT],
                            zstage[0:1, s0 * OUT : (s0 + n) * OUT],
                            accum_op=mybir.AluOpType.add,
                        )
                if ci == 0:
                    # W-pair tile for the fused pair multiplies (DVE
                    # tensor_copy runs in 2x_2P mode, ~0.7us per half; ACT
                    # would take 2us per half)
                    wr2_sb = cpool.tile([128, 2 * WT_COLS], bf16)
                    nc.vector.tensor_copy(wr2_sb[:, :WT_COLS], wr_sb[:])
                    nc.vector.tensor_copy(wr2_sb[:, WT_COLS:], wr_sb[:])
                if ci == 1:
                    # bias*Berr: computed on the otherwise-idle GpSimd engine
                    # and pre-stored into the output; the per-group stores
                    # above ACCUMULATE onto it via the SDMA CCE adder. All
                    # five stores share the single SWDGE queue, whose FIFO
                    # order guarantees the pre-store lands first (the first
                    # accumulating store is emitted under ci == 2).
                    nc.gpsimd.tensor_mul(memb_sb[:], berr_sb[:], bias_sb[:])
                    nc.gpsimd.dma_start(z.ap(), memb_sb[:])

    nc.compile()
    return nc


def get_nc():
    if "nc" not in _CACHE:
        _CACHE["nc"] = _build()
    return _CACHE["nc"]


def make_in_maps(X, W, bias, Werr, Berr, loc_id):
    X = np.ascontiguousarray(np.asarray(X, dtype=np.float32))
    W = np.ascontiguousarray(np.asarray(W, dtype=np.float32))
    bias = np.ascontiguousarray(np.asarray(bias, dtype=np.float32))
    Werr = np.asarray(Werr, dtype=np.float32)
    Berr = np.asarray(Berr, dtype=np.float32)
    loc_id = np.asarray(loc_id, dtype=np.int32)

    wrb = np.ascontiguousarray(W.reshape(128, WT_COLS).astype(BF16))
    bias16 = np.ascontiguousarray(np.broadcast_to(bias[None, :], (BL, OUT)))

    in_maps = []
    for c in range(NCORES):
        xc = X[c * BL : (c + 1) * BL]  # [BL, IN]
        locc = loc_id[c * BL : (c + 1) * BL]  # [BL]
        # slab b in columns [b*2048:(b+1)*2048]; partition p = in-rows 4p..4p+3
        wdc = np.ascontiguousarray(
            Werr[locc]
            .astype(BF16)
            .reshape(BL, 128, WT_COLS)
            .transpose(1, 0, 2)
            .reshape(128, BL * WT_COLS)
        )
        xtc = np.ascontiguousarray(
            xc.reshape(BL, 128, 4).transpose(1, 0, 2).reshape(128, BL * 4).astype(BF16)
        )
        in_maps.append(
            {
                "WD": wdc,
                "Wr": wrb,
                "Xt": xtc,
                "bias16": bias16,
                "berr16": np.ascontiguousarray(Berr[locc]),
            }
        )
    return in_maps


def _reset_accelerator():
    import ctypes

    try:
        lib = ctypes.CDLL("/opt/axon/libaxon_pjrt.so")
        lib.axon_reset.restype = ctypes.c_int64
        lib.axon_reset()
    except Exception:
        pass


def kernel(X, W, bias, Werr, Berr, loc_id):
    from concourse.bass_utils import run_bass_kernel_spmd

    nc = get_nc()
    in_maps = make_in_maps(X, W, bias, Werr, Berr, loc_id)
    try:
        res = run_bass_kernel_spmd(nc, in_maps, core_ids=list(range(NCORES)))
    except Exception:
        # a wedged NeuronCore surfaces as an unrecoverable-device error;
        # reset the accelerator once and retry
        _reset_accelerator()
        res = run_bass_kernel_spmd(nc, in_maps, core_ids=list(range(NCORES)))
    out = np.concatenate(
        [res.results[c]["Z"].reshape(BL, OUT) for c in range(NCORES)], axis=0
    )
    return out


# revision 48
# speedup vs baseline: 1.0551x; 1.0551x over previous
"""Bass/Trainium2 kernel for DropConnect (training path, Wstd != 0).

Z[b,o] = sum_i X[b,i] * W[i,o] * Werr[loc_id[b],i,o] + bias[o] * Berr[loc_id[b],o]

Strategy (8 NeuronCores, data-parallel over batch):
  - each core handles 16 samples; loc_id is known on the host at launch, so
    the per-sample Werr/Berr rows are gathered host-side while sharding and
    shipped per-core as plain contiguous inputs (the "all-gather of the
    needed rows" sharding choice) -- no on-device indirect DMA at all
  - slabs are shipped in bf16 (tolerance is 2e-2; measured end-to-end
    rel err ~3e-3), halving HBM traffic to ~8.4 MB/core
  - W and the whole slab stream ride ONE HWDGE ring (SP) in consumption
    order: per-ring FIFO guarantees arrival order with no packet-size
    competition (the DMA sustains ~425 GB/s after a ~5us ramp); the tiny
    xt/bias/berr inputs drain on the ACT ring
  - VectorE multiplies slab pairs with W in fused [128,4096] ops (2x_1P
    bf16 mode) -- the steady-state pacer at ~1.14us/sample; head/tail
    slabs are halved so compute starts/ends tighter against the stream
  - TensorE contracts with X: per sample, 4 accumulating [128,1]x[128,512]
    matmuls into a [1,512] PSUM tile (215ns issue cadence warm)
  - bias*Berr is computed on the otherwise-idle GpSimd and pre-stored to
    the output; ScalarE copies each PSUM row into a flat staging tile and
    the group stores ACCUMULATE onto the pre-store via the SDMA CCE adder
    (single SWDGE queue FIFO guarantees ordering)
"""

import sys

sys.path.insert(0, "/opt/trn_rl_repo")

import ml_dtypes
import numpy as np

B, IN, OUT, POOL, NCORES = 128, 512, 512, 1000, 8
BL = B // NCORES  # samples per core
WT_COLS = 4 * OUT  # 2048: one macro-row = 4 input rows of W/Werr

BF16 = ml_dtypes.bfloat16

# slab -> chunk grouping: two 1-slab chunks first (fast first compute),
# 1MB pair chunks in the middle, single-slab chunks last (short drain).
CHUNK_SLABS = [[0], [1], [2, 3], [4, 5], [6, 7], [8, 9], [10, 11], [12, 13], [14], [15]]

# sample index -> (first sample, count) for the accumulating output stores;
# the final two stores cover 2 samples each -- more stores than this loses:
# each SWDGE store costs ~0.6-0.9us of serialized Q7 issue time
STORE_AT = {3: (0, 4), 7: (4, 4), 11: (8, 4), 13: (12, 2), 15: (14, 2)}

_CACHE = {}


def _build():
    import concourse.mybir as mybir
    import concourse.tile as tile
    from concourse import bacc

    f32, bf16 = mybir.dt.float32, mybir.dt.bfloat16

    nc = bacc.Bacc("TRN2", debug=False)
    wd = nc.dram_tensor("WD", [128, BL * WT_COLS], bf16, kind="ExternalInput")
    wr = nc.dram_tensor("Wr", [128, WT_COLS + BL * 4], bf16, kind="ExternalInput")
    bias16 = nc.dram_tensor("bias16", [BL, OUT], f32, kind="ExternalInput")
    berr16 = nc.dram_tensor("berr16", [BL, OUT], f32, kind="ExternalInput")
    z = nc.dram_tensor("Z", [1, BL * OUT], f32, kind="ExternalOutput")

    with tile.TileContext(nc) as tc:
        with (
            tc.tile_pool(name="const", bufs=1) as cpool,
            tc.tile_pool(name="wts", bufs=8) as wpool,
            tc.tile_pool(name="prod", bufs=6) as ptpool,
            tc.tile_pool(name="ps", bufs=8, space="PSUM") as ppool,
        ):
            wt_tiles = {}

            def chunk_dma(ci, ring):
                slabs = CHUNK_SLABS[ci]
                w = len(slabs) * WT_COLS
                t = wpool.tile([128, 2 * WT_COLS], bf16, tag="wt")
                base = slabs[0] * WT_COLS
                if ci >= len(CHUNK_SLABS) - 2:
                    # tail slabs arrive as two half-DMAs so their multiplies
                    # and matmuls start before the full slab lands
                    h = w // 2
                    ring.dma_start(t[:, :h], wd.ap()[:, base : base + h])
                    ring.dma_start(t[:, h:w], wd.ap()[:, base + h : base + w])
                else:
                    ring.dma_start(t[:, :w], wd.ap()[:, base : base + w])
                wt_tiles[ci] = t

            # W and the ENTIRE slab stream ride the SP ring: FIFO order per
            # ring means W beats the flood and slabs arrive exactly in
            # consumption order with no packet competition. The tiny
            # xt/bias/berr inputs drain on the ACT ring; only the (idle)
            # GpSimd memb product and the first matmul's stationary read
            # depend on them.
            # W and the X columns ride ONE DMA (X packed into W's last 64
            # columns host-side; a separate 128B-per-line xt DMA is pure
            # descriptor overhead)
            wrx_sb = cpool.tile([128, WT_COLS + BL * 4], bf16)
            nc.sync.dma_start(wrx_sb[:], wr.ap())
            bias_sb = cpool.tile([BL, OUT], f32)
            nc.scalar.dma_start(bias_sb[:], bias16.ap())
            berr_sb = cpool.tile([BL, OUT], f32)
            nc.scalar.dma_start(berr_sb[:], berr16.ap())
            for ci in range(len(CHUNK_SLABS)):
                chunk_dma(ci, nc.sync)

            memb_sb = cpool.tile([BL, OUT], bf16)
            zstage = cpool.tile([1, BL * OUT], f32)

            for ci, slabs in enumerate(CHUNK_SLABS):
                wt = wt_tiles[ci]
                w = len(slabs) * WT_COLS
                pt = ptpool.tile([128, 2 * WT_COLS], bf16, tag="pt")
                if ci >= len(CHUNK_SLABS) - 2:
                    h = w // 2
                    nc.vector.tensor_mul(pt[:, :h], wt[:, :h], wrx_sb[:, :h])
                    nc.vector.tensor_mul(pt[:, h:w], wt[:, h:w], wrx_sb[:, h:w])
                elif len(slabs) == 1:
                    nc.vector.tensor_mul(pt[:, :w], wt[:, :w], wrx_sb[:, :WT_COLS])
                else:
                    nc.vector.tensor_mul(pt[:, :w], wt[:, :w], wr2_sb[:])
                for si, b in enumerate(slabs):
                    ps = ppool.tile([1, OUT], f32, tag="ps")
                    for j in range(4):
                        nc.tensor.matmul(
                            out=ps[:],
                            lhsT=wrx_sb[:, WT_COLS + 4 * b + j : WT_COLS + 4 * b + j + 1],
                            rhs=pt[:, si * WT_COLS + j * OUT : si * WT_COLS + (j + 1) * OUT],
                            start=(j == 0),
                            stop=(j == 3),
                        )
                    nc.scalar.copy(out=zstage[0:1, b * OUT : (b + 1) * OUT], in_=ps[:])
                    # accumulate finished rows onto the memb pre-store; the
                    # final two stores cover 2 samples each so the last store
                    # is small and issues as early as possible
                    if b in STORE_AT:
                        s0, n = STORE_AT[b]
                        nc.gpsimd.dma_start(
                            z.ap()[:, s0 * OUT : (s0 + n) * OUT],
                            zstage[0:1, s0 * OUT : (s0 + n) * OUT],
                            accum_op=mybir.AluOpType.add,
                        )
                if ci == 0:
                    # W-pair tile for the fused pair multiplies (DVE
                    # tensor_copy runs in 2x_2P mode, ~0.7us per half; ACT
                    # would take 2us per half)
                    wr2_sb = cpool.tile([128, 2 * WT_COLS], bf16)
                    nc.vector.tensor_copy(wr2_sb[:, :WT_COLS], wrx_sb[:, :WT_COLS])
                    nc.vector.tensor_copy(wr2_sb[:, WT_COLS:], wrx_sb[:, :WT_COLS])
                if ci == 1:
                    # bias*Berr: computed on the otherwise-idle GpSimd engine
                    # and pre-stored into the output; the per-group stores
                    # above ACCUMULATE onto it via the SDMA CCE adder. All
                    # five stores share the single SWDGE queue, whose FIFO
                    # order guarantees the pre-store lands first (the first
                    # accumulating store is emitted under ci == 2).
                    nc.gpsimd.tensor_mul(memb_sb[:], berr_sb[:], bias_sb[:])
                    nc.gpsimd.dma_start(z.ap(), memb_sb[:])

    nc.compile()
    return nc


def get_nc():
    if "nc" not in _CACHE:
        _CACHE["nc"] = _build()
    return _CACHE["nc"]


def make_in_maps(X, W, bias, Werr, Berr, loc_id):
    X = np.ascontiguousarray(np.asarray(X, dtype=np.float32))
    W = np.ascontiguousarray(np.asarray(W, dtype=np.float32))
    bias = np.ascontiguousarray(np.asarray(bias, dtype=np.float32))
    Werr = np.asarray(Werr, dtype=np.float32)
    Berr = np.asarray(Berr, dtype=np.float32)
    loc_id = np.asarray(loc_id, dtype=np.int32)

    wrb = W.reshape(128, WT_COLS).astype(BF16)
    bias16 = np.ascontiguousarray(np.broadcast_to(bias[None, :], (BL, OUT)))

    in_maps = []
    for c in range(NCORES):
        xc = X[c * BL : (c + 1) * BL]  # [BL, IN]
        locc = loc_id[c * BL : (c + 1) * BL]  # [BL]
        # slab b in columns [b*2048:(b+1)*2048]; partition p = in-rows 4p..4p+3
        wdc = np.ascontiguousarray(
            Werr[locc]
            .astype(BF16)
            .reshape(BL, 128, WT_COLS)
            .transpose(1, 0, 2)
            .reshape(128, BL * WT_COLS)
        )
        xtc = (
            xc.reshape(BL, 128, 4).transpose(1, 0, 2).reshape(128, BL * 4).astype(BF16)
        )
        wrxc = np.ascontiguousarray(np.concatenate([wrb, xtc], axis=1))
        in_maps.append(
            {
                "WD": wdc,
                "Wr": wrxc,
                "bias16": bias16,
                "berr16": np.ascontiguousarray(Berr[locc]),
            }
        )
    return in_maps


def _reset_accelerator():
    import ctypes

    try:
        lib = ctypes.CDLL("/opt/axon/libaxon_pjrt.so")
        lib.axon_reset.restype = ctypes.c_int64
        lib.axon_reset()
    except Exception:
        pass


def kernel(X, W, bias, Werr, Berr, loc_id):
    from concourse.bass_utils import run_bass_kernel_spmd

    nc = get_nc()
    in_maps = make_in_maps(X, W, bias, Werr, Berr, loc_id)
    try:
        res = run_bass_kernel_spmd(nc, in_maps, core_ids=list(range(NCORES)))
    except Exception:
        # a wedged NeuronCore surfaces as an unrecoverable-device error;
        # reset the accelerator once and retry
        _reset_accelerator()
        res = run_bass_kernel_spmd(nc, in_maps, core_ids=list(range(NCORES)))
    out = np.concatenate(
        [res.results[c]["Z"].reshape(BL, OUT) for c in range(NCORES)], axis=0
    )
    return out
